# revision 41
# baseline (speedup 1.0000x reference)
"""Trainium2 Bass kernel for C = tril(A @ B), A/B lower-triangular 4096x4096 fp32.

Distribution (SPMD, 8 cores = 4 row-groups x 2 col-groups): core (g, h) owns
row-blocks {4t+g : t=0..7} (slots) and columns {512*(2l+h) : l=0..3} (locals).
Slot t uses a uniform K bound of 4*(t+1) k-blocks and local col l a uniform
K start of 8*l so every core runs the identical program; inputs are exactly
triangular, so all over-computed terms are exact zeros (no masking needed).

Schedule: every pass is k-major. Pass 0 (l=0) sweeps k ascending over ALL
row-slots per 4-k-block B chunk, so the very first B chunk feeds 8 slots of
matmuls (max compute per early DMA byte — the startup transient is the only
regime where HBM can't keep up with the PE). A^T is host-packed k-slice-major
(slice cc = k-blocks 4cc..4cc+3 of every band t>=cc) so a slice is 1-2
strided dma_starts (each dma_start costs ~640ns of serial sequencer issue).
ALL input DMAs ride the sync HWDGE ring in consumption order ~2 rounds
ahead: the 16 hw queues drain descriptors in issue order and their FIFOs
backpressure the sequencers, so both over-prefetching (parks MBs of
descriptors in front of later chunks; bp bufs=16 or pre-issuing all slices
regresses ~12us) and under-buffering (bufs=10) lose — bufs=12 with the
explicit interleave below is tuned. Passes 1-3 are k-major per l; PSUM tag
ps{t-2l} reuses the bank freed EARLIEST by the previous pass (kills
cross-pass psum-reuse stalls). Output is float16 (rel-err gate is 2e-2):
evictions (DVE cast PSUM->SBUF, then DMA) ride the gpsimd SWDGE ring for
l<2, sync for l>=2 (sync is idle by then; measured faster than all-gpsimd
or all-sync). Head k-blocks of each group run with a tapered free dim
(128*(d+1)) since the col-tile is structurally zero beyond that; head B
chunk DMAs are taper-trimmed to 2 pieces. Matmul operands are float16
(fp32 PSUM accumulation); ~3.6e-4 rel err, ~75us on HW (from 81us
baseline; PE busy floor is ~57us = 128k cycles at ~2.24GHz, fixed
runtime preamble+drain ~10us).

Measured dead ends: fp8e4m3 DoubleRow runs at 1 cycle per output column
(216ns per 512-wide, same as fp16) — the 2-slot contraction gain is exactly
eaten by hi/lo-split redundancy, net 1.5x slower. fp32r doubles DMA bytes.
Scalar-ring A loads, wavefront ramp order, and at-band prefetch reorders
all regressed on HW.

Host repack (partition-major, contiguous per partition per DMA):
  - A^T k-slice-major pack [128, 144*128] (slice cc: bands cc..7, 4 k-blocks
    each), SBUF-resident as [128, 8, 32, 128] (bands padded to 32 k-blocks).
  - B col-band nonzero-triangle tiles as 4-k-block chunks [20, 128, 2048].
"""

import numpy as np

N = 4096
P = 128
NCORES = 8
RG, CG = 4, 2           # row groups x col groups
SLOTS = N // P // RG    # 8 row-block slots per core
L = N // 512 // CG      # 4 local 512-col tiles per core
KB = N // P             # 32 k-blocks
CW = 512                # matmul free dim (fp32 max)
KC = 4                  # k-blocks per B chunk

MM_DT_NAME = "float16"  # float32 | float32r | float16 | bfloat16

# A^T k-slice-major: slice cc holds bands t = cc..7, k-blocks 4cc..4cc+3
SL_KB = [(SLOTS - cc) * KC for cc in range(SLOTS)]    # k-blocks per slice
SL_OFF = [sum(SL_KB[:cc]) for cc in range(SLOTS)]
AT_TOT = sum(SL_KB)                                   # 144 k-blocks

B_CHUNKS = [(l, cc) for l in range(L) for cc in range((KB - 8 * l) // KC)]
B_CI = {(l, cc): i for i, (l, cc) in enumerate(B_CHUNKS)}

_cached = {}


def _build(mm_dt_name):
    import concourse.mybir as mybir
    import concourse.tile as tile
    from concourse import bacc

    mm_dt = getattr(mybir.dt, mm_dt_name)

    nc = bacc.Bacc("TRN2", target_bir_lowering=False, debug=False,
                   num_devices=NCORES)
    at_d = nc.dram_tensor("at", [P, AT_TOT * P], mm_dt,
                          kind="ExternalInput").ap()
    b_d = nc.dram_tensor("b", [len(B_CHUNKS) * P, KC * CW], mm_dt,
                         kind="ExternalInput").ap()
    o_dt = mybir.dt.float16 if mm_dt_name in ("float16", "bfloat16") \
        else mybir.dt.float32
    o_d = nc.dram_tensor("o", [SLOTS, P, L * CW], o_dt,
                         kind="ExternalOutput").ap()

    with tile.TileContext(nc) as tc:
        with (
            tc.tile_pool(name="atp", bufs=1) as atp,
            tc.tile_pool(name="bp", bufs=12) as bp,
            tc.tile_pool(name="pp", bufs=1, space="PSUM") as pp,
            tc.tile_pool(name="sp", bufs=3) as sp,
        ):
            # one SBUF-resident A^T tensor, bands padded to KB k-blocks for a
            # uniform band stride (slice DMA = one strided descriptor set)
            a4 = atp.tile([P, SLOTS, KB, P], mm_dt, tag="at", name="at4")
            chunks = {}

            def load_slice(cc, bands=None):
                # slice cc: bands t=cc..7, k-blocks 4cc..4cc+3; optionally a
                # band subrange (b0, b1) for finer first-slice dependencies
                b0, b1 = bands if bands else (cc, SLOTS)
                src0 = SL_OFF[cc] + (b0 - cc) * KC
                n = (b1 - b0) * KC
                nc.sync.dma_start(
                    a4[:, b0:b1, KC * cc:KC * (cc + 1), :],
                    at_d[:, src0 * P:(src0 + n) * P])

            def load_chunk(l, cc):
                ci = B_CI[(l, cc)]
                bch = bp.tile([P, KC, CW], mm_dt, tag="b", name=f"b{ci}")
                if cc == 0:
                    # head chunk: k-block q is structurally zero beyond
                    # 128*(q+1) cols. Two strided DMAs (issue time ~640ns
                    # each beats per-q trims): q0-1 at width 256, q2-3 full.
                    nc.sync.dma_start(
                        bch[:, 0:2, :256],
                        b_d[ci * P:(ci + 1) * P, :].rearrange(
                            "p (k w) -> p k w", k=KC)[:, 0:2, :256])
                    nc.sync.dma_start(
                        bch[:, 2:4, :],
                        b_d[ci * P:(ci + 1) * P, 2 * CW:])
                else:
                    nc.sync.dma_start(bch[:], b_d[ci * P:(ci + 1) * P, :])
                chunks[(l, cc)] = bch

            def evict(t, l, ps):
                st = sp.tile([P, CW], o_dt, tag="st",
                             name=f"st{t}_{l}")
                nc.vector.tensor_copy(st[:], ps[:])
                # final pass: sync ring is done with B loads and has lower
                # first-byte latency than SWDGE - shortens the exit tail
                eng = nc.sync if l >= L - 2 else nc.gpsimd
                eng.dma_start(o_d[t, :, l * CW:(l + 1) * CW], st[:])

            # PE p-state warmup: the PE idles ~5us for the first DMAs, then
            # ramps its clock (DVFS) during the real taper matmuls; burn the
            # wait on dummy matmuls so real work starts at full clock
            wt = sp.tile([P, CW], mm_dt, tag="warm", name="warm")
            nc.vector.memset(wt[:], 0)
            wps = pp.tile([P, CW], mybir.dt.float32, tag="ps7",
                          name="warm_ps")
            for i in range(12):
                nc.tensor.matmul(wps[:], lhsT=wt[:, :P], rhs=wt[:],
                                 start=True, stop=True)

            # ---- pass 0 (l=0): k-major over all slots ----
            # single ring, consumption order, ~2 rounds ahead: the queue FIFO
            # drains in issue order, so over-prefetching A slices would park
            # megabytes of descriptors in front of the B chunks
            load_slice(0, bands=(0, 2))     # 256KB: unblocks bands 0-1
            load_chunk(0, 0)                # head chunk, 2 trimmed DMAs
            load_slice(0, bands=(2, 4))
            load_slice(0, bands=(4, 6))
            load_slice(0, bands=(6, 8))
            load_chunk(0, 1)
            load_slice(1, bands=(1, 5))
            load_chunk(0, 2)
            load_slice(1, bands=(5, 8))
            load_slice(2, bands=(2, 5))
            load_chunk(0, 3)
            load_slice(2, bands=(5, 8))
            psums = {}
            for cc in range(KB // KC):
                if 2 <= cc and cc + 2 < KB // KC:
                    load_chunk(0, cc + 2)
                if 1 <= cc and cc + 2 < SLOTS:
                    load_slice(cc + 2)
                bch = chunks[(0, cc)]
                if cc == 0:
                    # ramp: band-major so compute starts after the first
                    # band piece + head chunk, not the whole 1MB slice
                    for t in range(SLOTS):
                        psums[t] = pp.tile([P, CW], mybir.dt.float32,
                                           tag=f"ps{t}", name=f"ps{t}_0")
                        for q in range(KC):
                            w = min(CW, P * (q + 1))
                            nc.tensor.matmul(
                                psums[t][:, :w], lhsT=a4[:, t, q, :],
                                rhs=bch[:, q, :w],
                                start=(q == 0), stop=(t == 0 and q == KC - 1))
                else:
                    for q in range(KC):
                        k = KC * cc + q
                        for t in range(cc, SLOTS):
                            nc.tensor.matmul(
                                psums[t][:], lhsT=a4[:, t, k, :],
                                rhs=bch[:, q, :],
                                start=False,
                                stop=(t == cc and q == KC - 1))
                evict(cc, 0, psums[cc])

            # ---- passes 1..3: k-major ----
            for l in range(1, L):
                psums = {}
                for cc in range((KB - 8 * l) // KC):
                    load_chunk(l, cc)
                    for q in range(KC):
                        k = 8 * l + KC * cc + q
                        for t in range(2 * l, SLOTS):
                            kend = RG * (t + 1)
                            if k >= kend:
                                continue
                            if k == 8 * l:
                                # tag ps{t-2l}: reuse the bank freed EARLIEST
                                # by the previous pass (no cross-pass stall)
                                psums[t] = pp.tile([P, CW], mybir.dt.float32,
                                                   tag=f"ps{t - 2 * l}",
                                                   name=f"ps{t}_{l}")
                            w = min(CW, P * (k - 8 * l + 1))
                            nc.tensor.matmul(
                                psums[t][:, :w],
                                lhsT=a4[:, t, k, :],
                                rhs=chunks[(l, cc)][:, q, :w],
                                start=(k == 8 * l),
                                stop=(k == kend - 1),
                            )
                            if k == kend - 1:
                                evict(t, l, psums[t])

    nc.compile()
    return nc


def _get_nc(mm_dt_name):
    if mm_dt_name not in _cached:
        _cached[mm_dt_name] = _build(mm_dt_name)
    return _cached[mm_dt_name]


def _np_dt(mm_dt_name):
    if mm_dt_name == "float16":
        return np.float16
    if mm_dt_name == "bfloat16":
        import ml_dtypes
        return ml_dtypes.bfloat16
    return np.float32


def _pack_b(B, h, np_dt=np.float32):
    """[20*128, 2048]: chunk (l, cc) row p = 4 k-tiles' (k = 8l+4cc ..) row p
    of global col-tile 2l+h, concatenated."""
    B = B.astype(np_dt)
    B4 = B.reshape(KB, P, N // CW, CW)
    slabs = []
    for l, cc in B_CHUNKS:
        ks = 8 * l + KC * cc
        slabs.append(
            B4[ks:ks + KC, :, 2 * l + h, :].transpose(1, 0, 2)
            .reshape(P, KC * CW))
    return np.ascontiguousarray(np.stack(slabs)).reshape(len(B_CHUNKS) * P,
                                                         KC * CW)


def _pack_at(A, g, np_dt=np.float32):
    """[128, 144*128] k-slice-major: slice cc = bands t=cc..7, each band's
    k-blocks 4cc..4cc+3 of A^T[block 4t+g], laid out (p, t, k, m)."""
    A = A.astype(np_dt)
    out = np.empty((P, AT_TOT * P), dtype=np_dt)
    col = 0
    for cc in range(SLOTS):
        for t in range(cc, SLOTS):
            blk = RG * t + g
            blockT = A[blk * P:(blk + 1) * P,
                       cc * KC * P:(cc + 1) * KC * P].T      # [512, 128]
            arr = blockT.reshape(KC, P, P).transpose(1, 0, 2)
            out[:, col:col + KC * P] = arr.reshape(P, KC * P)
            col += KC * P
    return out


def kernel(A, B, mm_dt_name=MM_DT_NAME, trace=False):
    from concourse.bass_utils import run_bass_kernel_spmd

    A = np.ascontiguousarray(np.asarray(A, dtype=np.float32))
    B = np.ascontiguousarray(np.asarray(B, dtype=np.float32))

    nc = _get_nc(mm_dt_name)
    np_dt = _np_dt(mm_dt_name)
    b_packs = [_pack_b(B, h, np_dt) for h in range(CG)]
    in_maps = [{"at": _pack_at(A, c % RG, np_dt), "b": b_packs[c // RG]}
               for c in range(NCORES)]

    res = None
    for attempt in range(3):
        try:
            res = run_bass_kernel_spmd(nc, in_maps,
                                       core_ids=list(range(NCORES)),
                                       trace=trace)
            break
        except Exception:
            if attempt == 2:
                raise
            import time
            time.sleep(2)
    C = np.zeros((N, N), dtype=np.float32)
    for c in range(NCORES):
        g, h = c % RG, c // RG
        o = res.results[c]["o"]
        for t in range(SLOTS):
            blk = RG * t + g
            for l in range(L):
                jt = 2 * l + h
                C[blk * P:(blk + 1) * P, jt * CW:(jt + 1) * CW] = \
                    o[t, :, l * CW:(l + 1) * CW]
    if trace:
        kernel.last_exec_time_ns = res.exec_time_ns
        kernel.last_results = res
    return C


# revision 42
# speedup vs baseline: 1.0124x; 1.0124x over previous
"""Trainium2 Bass kernel for C = tril(A @ B), A/B lower-triangular 4096x4096 fp32.

Distribution (SPMD, 8 cores = 4 row-groups x 2 col-groups): core (g, h) owns
row-blocks {4t+g : t=0..7} (slots) and columns {512*(2l+h) : l=0..3} (locals).
Slot t uses a uniform K bound of 4*(t+1) k-blocks and local col l a uniform
K start of 8*l so every core runs the identical program; inputs are exactly
triangular, so all over-computed terms are exact zeros (no masking needed).

Schedule: every pass is k-major. Pass 0 (l=0) sweeps k ascending over ALL
row-slots per 4-k-block B chunk, so the very first B chunk feeds 8 slots of
matmuls (max compute per early DMA byte — the startup transient is the only
regime where HBM can't keep up with the PE). A^T is host-packed k-slice-major
(slice cc = k-blocks 4cc..4cc+3 of every band t>=cc) so a slice is 1-2
strided dma_starts (each dma_start costs ~640ns of serial sequencer issue).
ALL input DMAs ride the sync HWDGE ring in consumption order ~2 rounds
ahead: the 16 hw queues drain descriptors in issue order and their FIFOs
backpressure the sequencers, so both over-prefetching (parks MBs of
descriptors in front of later chunks; bp bufs=16 or pre-issuing all slices
regresses ~12us) and under-buffering (bufs=10) lose — bufs=12 with the
explicit interleave below is tuned. Passes 1-3 are k-major per l; PSUM tag
ps{t-2l} reuses the bank freed EARLIEST by the previous pass (kills
cross-pass psum-reuse stalls). Output is float16 (rel-err gate is 2e-2):
evictions (DVE cast PSUM->SBUF, then DMA) ride the gpsimd SWDGE ring for
l<2, sync for l>=2 (sync is idle by then; measured faster than all-gpsimd
or all-sync). Head k-blocks of each group run with a tapered free dim
(128*(d+1)) since the col-tile is structurally zero beyond that; head B
chunk DMAs are taper-trimmed to 2 pieces. Matmul operands are float16
(fp32 PSUM accumulation); ~3.6e-4 rel err, ~75us on HW (from 81us
baseline; PE busy floor is ~57us = 128k cycles at ~2.24GHz, fixed
runtime preamble+drain ~10us).

Measured dead ends: fp8e4m3 DoubleRow runs at 1 cycle per output column
(216ns per 512-wide, same as fp16) — the 2-slot contraction gain is exactly
eaten by hi/lo-split redundancy, net 1.5x slower. fp32r doubles DMA bytes.
Scalar-ring A loads, wavefront ramp order, and at-band prefetch reorders
all regressed on HW.

Host repack (partition-major, contiguous per partition per DMA):
  - A^T k-slice-major pack [128, 144*128] (slice cc: bands cc..7, 4 k-blocks
    each), SBUF-resident as [128, 8, 32, 128] (bands padded to 32 k-blocks).
  - B col-band nonzero-triangle tiles as 4-k-block chunks [20, 128, 2048].
"""

import numpy as np

N = 4096
P = 128
NCORES = 8
RG, CG = 4, 2           # row groups x col groups
SLOTS = N // P // RG    # 8 row-block slots per core
L = N // 512 // CG      # 4 local 512-col tiles per core
KB = N // P             # 32 k-blocks
CW = 512                # matmul free dim (fp32 max)
KC = 4                  # k-blocks per B chunk

MM_DT_NAME = "float16"  # float32 | float32r | float16 | bfloat16

# A^T k-slice-major: slice cc holds bands t = cc..7, k-blocks 4cc..4cc+3
SL_KB = [(SLOTS - cc) * KC for cc in range(SLOTS)]    # k-blocks per slice
SL_OFF = [sum(SL_KB[:cc]) for cc in range(SLOTS)]
AT_TOT = sum(SL_KB)                                   # 144 k-blocks

B_CHUNKS = [(l, cc) for l in range(L) for cc in range((KB - 8 * l) // KC)]
B_CI = {(l, cc): i for i, (l, cc) in enumerate(B_CHUNKS)}

_cached = {}


def _build(mm_dt_name):
    import concourse.mybir as mybir
    import concourse.tile as tile
    from concourse import bacc

    mm_dt = getattr(mybir.dt, mm_dt_name)

    nc = bacc.Bacc("TRN2", target_bir_lowering=False, debug=False,
                   num_devices=NCORES)
    at_d = nc.dram_tensor("at", [P, AT_TOT * P], mm_dt,
                          kind="ExternalInput").ap()
    b_d = nc.dram_tensor("b", [len(B_CHUNKS) * P, KC * CW], mm_dt,
                         kind="ExternalInput").ap()
    o_dt = mybir.dt.float16 if mm_dt_name in ("float16", "bfloat16") \
        else mybir.dt.float32
    o_d = nc.dram_tensor("o", [SLOTS, P, L * CW], o_dt,
                         kind="ExternalOutput").ap()

    with tile.TileContext(nc) as tc:
        with (
            tc.tile_pool(name="atp", bufs=1) as atp,
            tc.tile_pool(name="bp", bufs=12) as bp,
            tc.tile_pool(name="pp", bufs=1, space="PSUM") as pp,
            tc.tile_pool(name="sp", bufs=3) as sp,
        ):
            # one SBUF-resident A^T tensor, bands padded to KB k-blocks for a
            # uniform band stride (slice DMA = one strided descriptor set)
            a4 = atp.tile([P, SLOTS, KB, P], mm_dt, tag="at", name="at4")
            chunks = {}

            def load_slice(cc, bands=None):
                # slice cc: bands t=cc..7, k-blocks 4cc..4cc+3; optionally a
                # band subrange (b0, b1) for finer first-slice dependencies
                b0, b1 = bands if bands else (cc, SLOTS)
                src0 = SL_OFF[cc] + (b0 - cc) * KC
                n = (b1 - b0) * KC
                nc.sync.dma_start(
                    a4[:, b0:b1, KC * cc:KC * (cc + 1), :],
                    at_d[:, src0 * P:(src0 + n) * P])

            def load_chunk(l, cc):
                ci = B_CI[(l, cc)]
                bch = bp.tile([P, KC, CW], mm_dt, tag="b", name=f"b{ci}")
                if cc == 0:
                    # head chunk: k-block q is structurally zero beyond
                    # 128*(q+1) cols. Two strided DMAs (issue time ~640ns
                    # each beats per-q trims): q0-1 at width 256, q2-3 full.
                    nc.sync.dma_start(
                        bch[:, 0:2, :256],
                        b_d[ci * P:(ci + 1) * P, :].rearrange(
                            "p (k w) -> p k w", k=KC)[:, 0:2, :256])
                    nc.sync.dma_start(
                        bch[:, 2:4, :],
                        b_d[ci * P:(ci + 1) * P, 2 * CW:])
                else:
                    nc.sync.dma_start(bch[:], b_d[ci * P:(ci + 1) * P, :])
                chunks[(l, cc)] = bch

            def evict(t, l, ps):
                st = sp.tile([P, CW], o_dt, tag="st",
                             name=f"st{t}_{l}")
                nc.vector.tensor_copy(st[:], ps[:])
                # final pass: sync ring is done with B loads and has lower
                # first-byte latency than SWDGE - shortens the exit tail
                eng = nc.sync if l >= L - 2 else nc.gpsimd
                eng.dma_start(o_d[t, :, l * CW:(l + 1) * CW], st[:])

            # PE p-state warmup: the PE idles ~5us for the first DMAs, then
            # ramps its clock (DVFS) during the real taper matmuls; burn the
            # wait on dummy matmuls so real work starts at full clock
            wt = sp.tile([P, CW], mm_dt, tag="warm", name="warm")
            nc.vector.memset(wt[:], 0)
            wps = pp.tile([P, CW], mybir.dt.float32, tag="ps7",
                          name="warm_ps")
            for i in range(9):
                nc.tensor.matmul(wps[:], lhsT=wt[:, :P], rhs=wt[:],
                                 start=True, stop=True)

            # ---- pass 0 (l=0): k-major over all slots ----
            # single ring, consumption order, ~2 rounds ahead: the queue FIFO
            # drains in issue order, so over-prefetching A slices would park
            # megabytes of descriptors in front of the B chunks
            load_slice(0, bands=(0, 2))     # 256KB: unblocks bands 0-1
            load_chunk(0, 0)                # head chunk, 2 trimmed DMAs
            load_slice(0, bands=(2, 4))
            load_slice(0, bands=(4, 6))
            load_slice(0, bands=(6, 8))
            load_chunk(0, 1)
            load_slice(1, bands=(1, 5))
            load_chunk(0, 2)
            load_slice(1, bands=(5, 8))
            load_slice(2, bands=(2, 5))
            load_chunk(0, 3)
            load_slice(2, bands=(5, 8))
            psums = {}
            for cc in range(KB // KC):
                if 2 <= cc and cc + 2 < KB // KC:
                    load_chunk(0, cc + 2)
                if 1 <= cc and cc + 2 < SLOTS:
                    load_slice(cc + 2)
                bch = chunks[(0, cc)]
                if cc == 0:
                    # ramp: band-major so compute starts after the first
                    # band piece + head chunk, not the whole 1MB slice
                    for t in range(SLOTS):
                        psums[t] = pp.tile([P, CW], mybir.dt.float32,
                                           tag=f"ps{t}", name=f"ps{t}_0")
                        for q in range(KC):
                            w = min(CW, P * (q + 1))
                            nc.tensor.matmul(
                                psums[t][:, :w], lhsT=a4[:, t, q, :],
                                rhs=bch[:, q, :w],
                                start=(q == 0), stop=(t == 0 and q == KC - 1))
                else:
                    for q in range(KC):
                        k = KC * cc + q
                        for t in range(cc, SLOTS):
                            nc.tensor.matmul(
                                psums[t][:], lhsT=a4[:, t, k, :],
                                rhs=bch[:, q, :],
                                start=False,
                                stop=(t == cc and q == KC - 1))
                evict(cc, 0, psums[cc])

            # ---- passes 1..3: k-major ----
            for l in range(1, L):
                psums = {}
                for cc in range((KB - 8 * l) // KC):
                    load_chunk(l, cc)
                    for q in range(KC):
                        k = 8 * l + KC * cc + q
                        for t in range(2 * l, SLOTS):
                            kend = RG * (t + 1)
                            if k >= kend:
                                continue
                            if k == 8 * l:
                                # tag ps{t-2l}: reuse the bank freed EARLIEST
                                # by the previous pass (no cross-pass stall)
                                psums[t] = pp.tile([P, CW], mybir.dt.float32,
                                                   tag=f"ps{t - 2 * l}",
                                                   name=f"ps{t}_{l}")
                            w = min(CW, P * (k - 8 * l + 1))
                            nc.tensor.matmul(
                                psums[t][:, :w],
                                lhsT=a4[:, t, k, :],
                                rhs=chunks[(l, cc)][:, q, :w],
                                start=(k == 8 * l),
                                stop=(k == kend - 1),
                            )
                            if k == kend - 1:
                                evict(t, l, psums[t])

    nc.compile()
    return nc


def _get_nc(mm_dt_name):
    if mm_dt_name not in _cached:
        _cached[mm_dt_name] = _build(mm_dt_name)
    return _cached[mm_dt_name]


def _np_dt(mm_dt_name):
    if mm_dt_name == "float16":
        return np.float16
    if mm_dt_name == "bfloat16":
        import ml_dtypes
        return ml_dtypes.bfloat16
    return np.float32


def _pack_b(B, h, np_dt=np.float32):
    """[20*128, 2048]: chunk (l, cc) row p = 4 k-tiles' (k = 8l+4cc ..) row p
    of global col-tile 2l+h, concatenated."""
    B = B.astype(np_dt)
    B4 = B.reshape(KB, P, N // CW, CW)
    slabs = []
    for l, cc in B_CHUNKS:
        ks = 8 * l + KC * cc
        slabs.append(
            B4[ks:ks + KC, :, 2 * l + h, :].transpose(1, 0, 2)
            .reshape(P, KC * CW))
    return np.ascontiguousarray(np.stack(slabs)).reshape(len(B_CHUNKS) * P,
                                                         KC * CW)


def _pack_at(A, g, np_dt=np.float32):
    """[128, 144*128] k-slice-major: slice cc = bands t=cc..7, each band's
    k-blocks 4cc..4cc+3 of A^T[block 4t+g], laid out (p, t, k, m)."""
    A = A.astype(np_dt)
    out = np.empty((P, AT_TOT * P), dtype=np_dt)
    col = 0
    for cc in range(SLOTS):
        for t in range(cc, SLOTS):
            blk = RG * t + g
            blockT = A[blk * P:(blk + 1) * P,
                       cc * KC * P:(cc + 1) * KC * P].T      # [512, 128]
            arr = blockT.reshape(KC, P, P).transpose(1, 0, 2)
            out[:, col:col + KC * P] = arr.reshape(P, KC * P)
            col += KC * P
    return out


def kernel(A, B, mm_dt_name=MM_DT_NAME, trace=False):
    from concourse.bass_utils import run_bass_kernel_spmd

    A = np.ascontiguousarray(np.asarray(A, dtype=np.float32))
    B = np.ascontiguousarray(np.asarray(B, dtype=np.float32))

    nc = _get_nc(mm_dt_name)
    np_dt = _np_dt(mm_dt_name)
    b_packs = [_pack_b(B, h, np_dt) for h in range(CG)]
    in_maps = [{"at": _pack_at(A, c % RG, np_dt), "b": b_packs[c // RG]}
               for c in range(NCORES)]

    res = None
    for attempt in range(3):
        try:
            res = run_bass_kernel_spmd(nc, in_maps,
                                       core_ids=list(range(NCORES)),
                                       trace=trace)
            break
        except Exception:
            if attempt == 2:
                raise
            import time
            time.sleep(2)
    C = np.zeros((N, N), dtype=np.float32)
    for c in range(NCORES):
        g, h = c % RG, c // RG
        o = res.results[c]["o"]
        for t in range(SLOTS):
            blk = RG * t + g
            for l in range(L):
                jt = 2 * l + h
                C[blk * P:(blk + 1) * P, jt * CW:(jt + 1) * CW] = \
                    o[t, :, l * CW:(l + 1) * CW]
    if trace:
        kernel.last_exec_time_ns = res.exec_time_ns
        kernel.last_results = res
    return C
